# revision 1
# baseline (speedup 1.0000x reference)
"""Physics-Attention Structured Mesh 3D — Trainium2 Bass kernel, 8 NeuronCores.

Sharding: spatial-parallel over the first grid axis (s0: 32 planes -> 8 slabs
of 4 planes each, halo 1 plane per side, zero-padded on the host). The
point-axis softmax is made shift-invariant with a static exponent shift M0,
so the only cross-core traffic is one AllReduce-add of the (E, S)
accumulators (66 KB) per batch element.

Per core pipeline (per batch element b):
  conv   : 27-tap shifted matmul, fp32r, stationary weights -> xp [512, 4096]
  logits : xp chunks (fp32) @ [slice_w|ada_w] -> lt wide tiles (n-layout)
  temp   : t = 0.5+clip(v), lt = l * recip(t)   (fp32 DVE)
  exp    : e = exp(lt - M0) (ACT) -> fp32r
  E/S    : E = e^T @ xp^T (PE transposes + fp32r matmuls), S = sum_n e
  AllReduce-add(E||S) ; F = E * recip(S)^2 / (1+1e-5)
  recon  : out_x = F_blkdiag @ e_g ; proj: out = out_x @ out_w^T + out_b
"""
import numpy as np

import concourse.bacc as bacc
import concourse.mybir as mybir
import concourse.tile as tile
from concourse.bass_utils import run_bass_kernel_spmd
from concourse.masks import make_identity

F32 = mybir.dt.float32
F32R = mybir.dt.float32r
ALU = mybir.AluOpType
AX = mybir.AxisListType

NCORES = 8
B = 4
C = 256
INNER = 512
HH = 32
NLOC = 4 * HH * HH          # 4096 points per core
NCH = NLOC // 128           # 32 n-chunks
GROUPS = 2                  # co-tile pair-groups
M0 = 60.0                   # static softmax exponent shift
LNLE = float(np.log(-np.log(np.float32(1e-6))))

_CACHE = {}


def _build():
    nc = bacc.Bacc("TRN2", target_bir_lowering=False, debug=False,
                   num_devices=NCORES)

    xt_d = nc.declare_dram_parameter("xt", [2, 128, B, 6, 34, 34], F32, isOutput=False)
    wst_d = nc.declare_dram_parameter("wst", [128, 54, 4, 128], F32, isOutput=False)
    cb_d = nc.declare_dram_parameter("cb", [128, 4], F32, isOutput=False)
    w2_d = nc.declare_dram_parameter("w2", [128, 66], F32, isOutput=False)
    brow_d = nc.declare_dram_parameter("brow", [1, 264], F32, isOutput=False)
    ow_d = nc.declare_dram_parameter("ow", [128, 8, 128], F32, isOutput=False)
    ob_d = nc.declare_dram_parameter("ob", [128, 2], F32, isOutput=False)
    out_d = nc.declare_dram_parameter("out", [B, 256, NLOC], F32, isOutput=True)

    es_in = [nc.dram_tensor(f"es_in{b}", [128, 130], F32) for b in range(B)]
    es_out = [nc.dram_tensor(f"es_out{b}", [128, 130], F32, addr_space="Shared")
              for b in range(B)]
    env = dict(xt_d=xt_d, wst_d=wst_d, cb_d=cb_d, w2_d=w2_d, brow_d=brow_d,
               ow_d=ow_d, ob_d=ob_d, out_d=out_d, es_in=es_in, es_out=es_out)

    with tile.TileContext(nc) as tc, \
         tc.tile_pool(name="konst", bufs=1) as konst, \
         tc.tile_pool(name="wstr", bufs=2) as wstrp, \
         tc.tile_pool(name="slab", bufs=8) as slabp, \
         tc.tile_pool(name="lts", bufs=4) as ltsp, \
         tc.tile_pool(name="est", bufs=4) as estp, \
         tc.tile_pool(name="egt", bufs=2) as egtp, \
         tc.tile_pool(name="xpc", bufs=4) as xpcp, \
         tc.tile_pool(name="xpt", bufs=2) as xptp, \
         tc.tile_pool(name="small", bufs=4) as small, \
         tc.tile_pool(name="oxw", bufs=4) as oxwp, \
         tc.tile_pool(name="outs", bufs=3) as outsp, \
         tc.tile_pool(name="dram", bufs=8, space="DRAM") as dramp, \
         tc.tile_pool(name="psA", bufs=2, space="PSUM") as psA, \
         tc.tile_pool(name="psL", bufs=2, space="PSUM") as psL, \
         tc.tile_pool(name="psT", bufs=2, space="PSUM") as psT, \
         tc.tile_pool(name="psE", bufs=2, space="PSUM") as psE:

        _emit(nc, tc, env, konst, wstrp, slabp, ltsp, estp, egtp, xpcp, xptp, small, oxwp, outsp, dramp, psA, psL, psT, psE)

    nc.compile()
    return nc


def _emit(nc, tc, env, konst, wstrp, slabp, ltsp, estp, egtp, xpcp, xptp, small, oxwp, outsp, dramp, psA, psL, psT, psE):
    xt_d, wst_d, cb_d, w2_d, brow_d = (env[k] for k in
        ("xt_d", "wst_d", "cb_d", "w2_d", "brow_d"))
    ow_d, ob_d, out_d, es_in, es_out = (env[k] for k in
        ("ow_d", "ob_d", "out_d", "es_in", "es_out"))

    # ---- constants ----
    cbt = konst.tile([128, 4], F32, tag="cbt")
    nc.sync.dma_start(cbt[:], cb_d.ap())
    w2t = konst.tile([128, 66], F32, tag="w2t")
    nc.sync.dma_start(w2t[:], w2_d.ap())
    biasbc = konst.tile([128, 2, 4, 33], F32, tag="biasbc")
    nc.sync.dma_start(biasbc[:].rearrange("p a b c -> p (a b c)"),
                      brow_d.ap().to_broadcast((128, 264)))
    ident = konst.tile([128, 128], F32, tag="ident")
    make_identity(nc, ident)
    identr = konst.tile([128, 128], F32R, tag="identr")
    nc.vector.tensor_copy(identr[:], ident[:])
    owt = konst.tile([128, 8, 128], F32R, tag="owt")
    nc.sync.dma_start(owt[:], ow_d.ap().bitcast(F32R))
    obt = konst.tile([128, 2], F32, tag="obt")
    nc.sync.dma_start(obt[:], ob_d.ap())
    m0b = konst.tile([128, 1], F32, tag="m0b")
    nc.vector.memset(m0b[:], -M0)
    zero128 = konst.tile([128, 128], F32, tag="zero128")
    nc.vector.memset(zero128[:], 0.0)

    xpd = [[dramp.tile([128, NLOC], F32, tag="xpd", name=f"xpd_{b_}_{c_}")
            for c_ in range(4)] for b_ in range(B)]

    for b in range(B):
        # ================= conv (cot-outer, streamed weights) =========
        for cot in range(4):
            wa = wstrp.tile([128, 27, 128], F32R, tag="wa",
                            name=f"wa_{b}_{cot}")
            nc.sync.dma_start(wa[:], wst_d.ap()[:, 0:27, cot, :].bitcast(F32R))
            wb = wstrp.tile([128, 27, 128], F32R, tag="wb",
                            name=f"wb_{b}_{cot}")
            nc.sync.dma_start(wb[:], wst_d.ap()[:, 27:54, cot, :].bitcast(F32R))
            sl = {}
            for po in range(4):
                for d0 in range(3):
                    pl = po + d0
                    for ch in range(2):
                        if (ch, pl) not in sl:
                            st = slabp.tile([128, 34, 34], F32R, tag="sl",
                                            name=f"sl_{b}_{cot}_{ch}_{pl}")
                            nc.sync.dma_start(
                                st[:], xt_d.ap()[ch, :, b, pl, :, :].bitcast(F32R))
                            sl[(ch, pl)] = st
                for hf in range(2):
                    ps = psA.tile([128, 512], F32, tag="cps")
                    for k in range(54):
                        t, ch = k // 2, k % 2
                        d0, d1, d2 = t // 9, (t // 3) % 3, t % 3
                        wt_ = wa if k < 27 else wb
                        rhs = sl[(ch, po + d0)][
                            :, 16 * hf + d1:16 * hf + d1 + 16, d2:d2 + 32]
                        nc.tensor.matmul(
                            ps[:], wt_[:, k % 27, :], rhs,
                            start=(k == 0), stop=(k == 53))
                    xpo = outsp.tile([128, 512], F32, tag="xpo")
                    nc.vector.tensor_scalar_add(xpo[:], ps[:], cbt[:, cot:cot + 1])
                    n0 = po * 1024 + hf * 512
                    nc.sync.dma_start(xpd[b][cot][:, n0:n0 + 512], xpo[:])

        # ====== logits/temp/exp strips + transposes + E matmuls ======
        egt = [egtp.tile([128, NLOC], F32R, tag="egt", name=f"egt_{b}_{g_}")
               for g_ in range(GROUPS)]
        eps = [psE.tile([128, 512], F32, tag="eps", name=f"eps_{b}_{g_}")
               for g_ in range(GROUPS)]
        for jj in range(0, NCH, 2):
            ests = []
            for g in range(GROUPS):
                ps = psL.tile([128, 2, 132], F32, tag="lps")
                for dj in range(2):
                    for a in range(2):
                        xpc = xpcp.tile([128, 128], F32, tag="xpc")
                        nc.sync.dma_start(
                            xpc[:],
                            xpd[b][2 * g + a][:, 128 * (jj + dj):128 * (jj + dj) + 128])
                        nc.tensor.matmul(ps[:, dj, 66 * a:66 * a + 66],
                                         xpc[:], w2t[:], start=True, stop=True)
                lts = ltsp.tile([128, 2, 4, 33], F32, tag="lts")
                nc.vector.tensor_tensor(
                    lts[:], ps[:].rearrange("p a (b c) -> p a b c", b=4, c=33),
                    biasbc[:], ALU.add)
                tt = small.tile([128, 2, 4], F32, tag="tt")
                nc.vector.tensor_scalar(tt[:], lts[:, :, :, 32],
                                        0.4, -0.4, ALU.min, ALU.max)
                nc.vector.tensor_scalar_add(tt[:], tt[:], 0.5)
                rt = small.tile([128, 2, 4], F32, tag="rt")
                nc.vector.reciprocal(rt[:], tt[:])
                est = estp.tile([128, 2, 4, 32], F32R, tag="est")
                nc.vector.tensor_tensor(
                    lts[:, :, :, 0:32], lts[:, :, :, 0:32],
                    rt[:].to_broadcast((128, 2, 4, 32)), ALU.mult)
                nc.scalar.activation(est[:], lts[:, :, :, 0:32],
                                     mybir.ActivationFunctionType.Exp,
                                     bias=m0b[:], scale=1.0)
                ests.append(est)
            for dj in range(2):
                j = jj + dj
                xpt = xptp.tile([128, 512], F32R, tag="xpt")
                for cot in range(4):
                    xpc2 = xpcp.tile([128, 128], F32R, tag="xpc2")
                    nc.sync.dma_start(
                        xpc2[:],
                        xpd[b][cot][:, 128 * j:128 * j + 128].bitcast(F32R))
                    pt = psT.tile([128, 128], F32R, tag="tps")
                    nc.tensor.transpose(pt[:], xpc2[:], identr[:])
                    nc.vector.tensor_copy(xpt[:, 128 * cot:128 * cot + 128], pt[:])
                for g in range(GROUPS):
                    echunk = ests[g][:, dj, :, :].rearrange("p b c -> p (b c)")
                    pe_t = psT.tile([128, 128], F32R, tag="tps")
                    nc.tensor.transpose(pe_t[:], echunk, identr[:])
                    nc.vector.tensor_copy(egt[g][:, 128 * j:128 * j + 128],
                                          pe_t[:])
                    nc.tensor.matmul(eps[g][:], echunk, xpt[:],
                                     start=(j == 0), stop=(j == NCH - 1))

        # ---- pack E-diag + S ; AllReduce ----
        es2 = small.tile([128, 130], F32, tag="es2")
        for g in range(GROUPS):
            ssum = small.tile([128, 1], F32, tag="ssum")
            nc.vector.reduce_sum(ssum[:], egt[g][:], axis=AX.X)
            nc.vector.tensor_copy(es2[:, 65 * g + 64:65 * g + 65], ssum[:])
            for k in range(4):
                nc.vector.tensor_copy(
                    es2[32 * k:32 * k + 32, 65 * g:65 * g + 64],
                    eps[g][32 * k:32 * k + 32,
                           64 * (4 * g + k):64 * (4 * g + k) + 64])
        nc.sync.dma_start(es_in[b].ap(), es2[:])
        nc.gpsimd.collective_compute(
            "AllReduce", ALU.add,
            ins=[es_in[b].ap()], outs=[es_out[b].ap()],
            replica_groups=[list(range(NCORES))])
        esr = small.tile([128, 130], F32, tag="esr")
        nc.sync.dma_start(esr[:], es_out[b].ap())

        # ---- F = E * recip(S)^2 / (1+1e-5) ----
        fts = []
        for g in range(GROUPS):
            r1 = small.tile([128, 1], F32, tag="r1")
            nc.vector.reciprocal(r1[:], esr[:, 65 * g + 64:65 * g + 65])
            ft1 = small.tile([128, 64], F32, tag="ft1")
            nc.vector.tensor_scalar_mul(ft1[:], esr[:, 65 * g:65 * g + 64], r1[:])
            ft = small.tile([128, 64], F32R, tag="ft")
            nc.vector.tensor_scalar(ft[:], ft1[:], r1[:], 1.0 / (1.0 + 1e-5),
                                    ALU.mult, ALU.mult)
            fts.append(ft)

        fbs = []
        for g in range(GROUPS):
            fb = small.tile([128, 128], F32R, tag="fb", name=f"fb_{b}_{g}")
            nc.vector.tensor_copy(fb[:], zero128[:])
            for a in range(2):
                nc.sync.dma_start(fb[64 * a:64 * a + 32, 0:64],
                                  fts[g][64 * a:64 * a + 32, :])
                nc.sync.dma_start(fb[64 * a + 32:64 * a + 64, 64:128],
                                  fts[g][64 * a + 32:64 * a + 64, :])
            fbs.append(fb)

        # ---- reconstruct + final projection ----
        for w in range(8):
            oxs = []
            for p in range(4):
                g, a = p // 2, p % 2
                pr = psA.tile([128, 512], F32, tag="cps")
                nc.tensor.matmul(pr[:], fbs[g][64 * a:64 * a + 64, :],
                                 egt[g][64 * a:64 * a + 64,
                                        512 * w:512 * w + 512],
                                 start=True, stop=True)
                ox = oxwp.tile([128, 512], F32R, tag="ox")
                nc.vector.tensor_copy(ox[:], pr[:])
                oxs.append(ox)
            for mt in range(2):
                po = psL.tile([128, 512], F32, tag="lps")
                for p in range(4):
                    nc.tensor.matmul(po[:], owt[:, 2 * p + mt, :], oxs[p][:],
                                     start=(p == 0), stop=(p == 3))
                osb = outsp.tile([128, 512], F32, tag="osb")
                nc.vector.tensor_scalar_add(osb[:], po[:], obt[:, mt:mt + 1])
                nc.sync.dma_start(
                    out_d.ap()[b, 128 * mt:128 * mt + 128,
                               512 * w:512 * w + 512],
                    osb[:])


def _prep_inputs(x, conv_w, conv_b, slice_w, slice_b, ada_w, ada_b, out_w, out_b):
    """Shard/transpose/pad the full inputs into 8 per-core input maps."""
    x = np.ascontiguousarray(x, np.float32)
    xT = np.zeros((C, B, 34, 34, 34), np.float32)
    xT[:, :, 1:33, 1:33, 1:33] = x.reshape(B, HH, HH, HH, C).transpose(4, 0, 1, 2, 3)

    # conv weights: [co, ci, 3,3,3] -> [ci%128, tap, ci//128, co]
    # [ci%128, k=(t*2+ch), cot, co%128]
    wst = np.ascontiguousarray(
        conv_w.reshape(INNER, C, 27).transpose(1, 2, 0)      # [ci, t, co]
              .reshape(2, 128, 27, 4, 128)                   # [ch, ci, t, cot, co]
              .transpose(1, 2, 0, 3, 4)                      # [ci, t, ch, cot, co]
              .reshape(128, 54, 4, 128), np.float32)
    cb = np.ascontiguousarray(conv_b.reshape(4, 128).T, np.float32)

    w2 = np.zeros((128, 66), np.float32)
    w2[0:64, 0:32] = slice_w.T
    w2[0:64, 32] = ada_w[0]
    w2[64:128, 33:65] = slice_w.T
    w2[64:128, 65] = ada_w[0]

    bvec = np.concatenate([slice_b - LNLE, ada_b]).astype(np.float32)  # [33]
    brow = np.tile(bvec, 8).reshape(1, 264)

    ow = np.ascontiguousarray(
        out_w.T.reshape(4, 128, 2, 128).transpose(1, 0, 2, 3), np.float32) \
        .reshape(128, 8, 128)
    ob = np.ascontiguousarray(out_b.reshape(2, 128).T, np.float32)

    in_maps = []
    for i in range(NCORES):
        slab = np.ascontiguousarray(xT[:, :, 4 * i:4 * i + 6, :, :]) \
            .reshape(2, 128, B, 6, 34, 34)
        in_maps.append({"xt": slab, "wst": wst, "cb": cb, "w2": w2,
                        "brow": brow, "ow": ow, "ob": ob})
    return in_maps


def kernel(**inputs):
    if "nc" not in _CACHE:
        _CACHE["nc"] = _build()
    nc = _CACHE["nc"]
    in_maps = _prep_inputs(
        np.asarray(inputs["x"]), np.asarray(inputs["conv_w"]),
        np.asarray(inputs["conv_b"]), np.asarray(inputs["slice_w"]),
        np.asarray(inputs["slice_b"]), np.asarray(inputs["ada_w"]),
        np.asarray(inputs["ada_b"]), np.asarray(inputs["out_w"]),
        np.asarray(inputs["out_b"]))
    res = run_bass_kernel_spmd(nc, in_maps, core_ids=list(range(NCORES)))
    out = np.empty((B, 32768, 256), np.float32)
    for i in range(NCORES):
        o = res.results[i]["out"]            # [B, 256, 4096]
        out[:, 4096 * i:4096 * (i + 1), :] = o.transpose(0, 2, 1)
    return out



# revision 3
# speedup vs baseline: 1.2551x; 1.2551x over previous
"""Physics-Attention Structured Mesh 3D — Trainium2 Bass kernel, 8 NeuronCores.

Sharding: spatial-parallel over the first grid axis (s0: 32 planes -> 8 slabs
of 4 planes each, halo 1 plane per side, zero-padded on the host). The
point-axis softmax is made shift-invariant with a static exponent shift M0,
so the only cross-core traffic is one AllReduce-add of the (E, S)
accumulators (66 KB) per batch element.

v2: the emission interleaves the post-conv phases (logits/temp/exp,
transposes, E accumulation, AllReduce, reconstruction) between the conv's
PSUM accumulation groups so the PE instruction stream never drains. This
keeps the HAM clock gate at its sustained level (13/16) instead of
dropping to 4/8 during the mixed phases, and hides the AllReduce latency
under the next batch's conv. PSUM->SBUF copies ride the scalar engine
(otherwise-idle) so the vector engine keeps up inside the interleave
windows.

Per core pipeline (per batch element b):
  conv   : 27-tap shifted matmul, fp32r, 32 PSUM groups -> xp [512, 4096]
  logits : xp chunks (fp32) @ [slice_w|ada_w] -> lt wide tiles (n-layout)
  temp   : t = 0.5+clip(v), lt = l * recip(t)   (fp32 DVE)
  exp    : e = exp(lt - M0) (ACT) -> fp32r
  E/S    : E = e^T @ xp^T (PE transposes + fp32r matmuls), S = sum_n e
  AllReduce-add(E||S) ; F = E * recip(S)^2 / (1+1e-5)   (bf16)
  recon  : out_x = F_blkdiag @ e_g (bf16) ; proj: out = out_x @ out_w^T + out_b
"""
import numpy as np

import concourse.bacc as bacc
import concourse.mybir as mybir
import concourse.tile as tile
from concourse.bass_utils import run_bass_kernel_spmd
from concourse.masks import make_identity

F32 = mybir.dt.float32
F32R = mybir.dt.float32r
BF16 = mybir.dt.bfloat16
ALU = mybir.AluOpType
AX = mybir.AxisListType
ACTF = mybir.ActivationFunctionType

NCORES = 8
B = 4
C = 256
INNER = 512
HH = 32
NLOC = 4 * HH * HH          # 4096 points per core
NCH = NLOC // 128           # 32 n-chunks
GROUPS = 2                  # co-tile pair-groups
M0 = 60.0                   # static softmax exponent shift
LNLE = float(np.log(-np.log(np.float32(1e-6))))

_CACHE = {}


def _build():
    nc = bacc.Bacc("TRN2", target_bir_lowering=False, debug=False,
                   num_devices=NCORES)

    xt_d = nc.declare_dram_parameter("xt", [2, 128, B, 6, 34, 34], F32, isOutput=False)
    wst_d = nc.declare_dram_parameter("wst", [128, 54, 4, 128], F32, isOutput=False)
    cb_d = nc.declare_dram_parameter("cb", [128, 4], F32, isOutput=False)
    w2_d = nc.declare_dram_parameter("w2", [128, 66], F32, isOutput=False)
    brow_d = nc.declare_dram_parameter("brow", [1, 264], F32, isOutput=False)
    ow_d = nc.declare_dram_parameter("ow", [128, 8, 128], F32, isOutput=False)
    ob_d = nc.declare_dram_parameter("ob", [128, 2], F32, isOutput=False)
    out_d = nc.declare_dram_parameter("out", [B, 256, NLOC], F32, isOutput=True)

    es_in = [nc.dram_tensor(f"es_in{b}", [128, 130], F32) for b in range(B)]
    es_out = [nc.dram_tensor(f"es_out{b}", [128, 130], F32, addr_space="Shared")
              for b in range(B)]
    env = dict(xt_d=xt_d, wst_d=wst_d, cb_d=cb_d, w2_d=w2_d, brow_d=brow_d,
               ow_d=ow_d, ob_d=ob_d, out_d=out_d, es_in=es_in, es_out=es_out)

    with tile.TileContext(nc) as tc, \
         tc.tile_pool(name="konst", bufs=1) as konst, \
         tc.tile_pool(name="wstr", bufs=2) as wstrp, \
         tc.tile_pool(name="slab", bufs=12) as slabp, \
         tc.tile_pool(name="lts", bufs=4) as ltsp, \
         tc.tile_pool(name="est", bufs=4) as estp, \
         tc.tile_pool(name="egt", bufs=4) as egtp, \
         tc.tile_pool(name="xpc", bufs=16) as xpcp, \
         tc.tile_pool(name="xpt", bufs=2) as xptp, \
         tc.tile_pool(name="small", bufs=4) as small, \
         tc.tile_pool(name="oxw", bufs=4) as oxwp, \
         tc.tile_pool(name="outs", bufs=3) as outsp, \
         tc.tile_pool(name="dram", bufs=8, space="DRAM") as dramp, \
         tc.tile_pool(name="psA", bufs=2, space="PSUM") as psA, \
         tc.tile_pool(name="psL", bufs=2, space="PSUM") as psL, \
         tc.tile_pool(name="psT", bufs=2, space="PSUM") as psT, \
         tc.tile_pool(name="psE", bufs=2, space="PSUM") as psE:

        pools = dict(konst=konst, wstrp=wstrp, slabp=slabp, ltsp=ltsp,
                     estp=estp, egtp=egtp, xpcp=xpcp, xptp=xptp, small=small,
                     oxwp=oxwp, outsp=outsp, dramp=dramp, psA=psA, psL=psL,
                     psT=psT, psE=psE)
        _emit(nc, tc, env, pools)

    nc.compile()
    return nc


def _emit(nc, tc, env, P):
    xt_d, wst_d, cb_d, w2_d, brow_d = (env[k] for k in
        ("xt_d", "wst_d", "cb_d", "w2_d", "brow_d"))
    ow_d, ob_d, out_d, es_in, es_out = (env[k] for k in
        ("ow_d", "ob_d", "out_d", "es_in", "es_out"))
    konst, wstrp, slabp, ltsp, estp = (P[k] for k in
        ("konst", "wstrp", "slabp", "ltsp", "estp"))
    egtp, xpcp, xptp, small, oxwp, outsp, dramp = (P[k] for k in
        ("egtp", "xpcp", "xptp", "small", "oxwp", "outsp", "dramp"))
    psA, psL, psT, psE = (P[k] for k in ("psA", "psL", "psT", "psE"))

    # ---- constants ----
    cbt = konst.tile([128, 4], F32, tag="cbt")
    nc.sync.dma_start(cbt[:], cb_d.ap())
    w2t = konst.tile([128, 66], F32, tag="w2t")
    nc.sync.dma_start(w2t[:], w2_d.ap())
    biasbc = konst.tile([128, 2, 4, 33], F32, tag="biasbc")
    nc.sync.dma_start(biasbc[:].rearrange("p a b c -> p (a b c)"),
                      brow_d.ap().to_broadcast((128, 264)))
    ident = konst.tile([128, 128], F32, tag="ident")
    make_identity(nc, ident)
    identr = konst.tile([128, 128], F32R, tag="identr")
    nc.vector.tensor_copy(identr[:], ident[:])
    owt = konst.tile([128, 8, 128], F32R, tag="owt")
    nc.sync.dma_start(owt[:], ow_d.ap().bitcast(F32R))
    obt = konst.tile([128, 2], F32, tag="obt")
    nc.sync.dma_start(obt[:], ob_d.ap())
    m0b = konst.tile([128, 1], F32, tag="m0b")
    nc.vector.memset(m0b[:], -M0)
    zerob = konst.tile([128, 128], BF16, tag="zerob")
    nc.vector.memset(zerob[:], 0.0)

    xpd = [[dramp.tile([128, NLOC], F32, tag="xpd", name=f"xpd_{b_}_{c_}")
            for c_ in range(4)] for b_ in range(B)]

    # ---------------- emission helpers -------------------------------
    wtab = {}    # cot_global -> (wa, wb)
    sltab = {}   # b -> {(ch, pl): slab tile}
    st2 = {}     # b -> dict(egt=[...], eps=[...])

    def load_weights(cg):
        b_, cot = cg // 4, cg % 4
        wa = wstrp.tile([128, 27, 128], F32R, tag="wa", name=f"wa_{b_}_{cot}")
        nc.sync.dma_start(wa[:], wst_d.ap()[:, 0:27, cot, :].bitcast(F32R))
        wb = wstrp.tile([128, 27, 128], F32R, tag="wb", name=f"wb_{b_}_{cot}")
        nc.sync.dma_start(wb[:], wst_d.ap()[:, 27:54, cot, :].bitcast(F32R))
        wtab[cg] = (wa, wb)

    def load_slabs(b):
        sl = {}
        for pl in range(6):
            for ch in range(2):
                st = slabp.tile([128, 34, 34], F32R, tag="sl",
                                name=f"sl_{b}_{ch}_{pl}")
                nc.sync.dma_start(
                    st[:], xt_d.ap()[ch, :, b, pl, :, :].bitcast(F32R))
                sl[(ch, pl)] = st
        sltab[b] = sl

    def conv_group(b, g):
        cot, po, hf = g // 8, (g % 8) // 2, g % 2
        wa, wb = wtab[4 * b + cot]
        sl = sltab[b]
        ps = psA.tile([128, 512], F32, tag="cps")
        for k in range(54):
            t, ch = k // 2, k % 2
            d0, d1, d2 = t // 9, (t // 3) % 3, t % 3
            wt_ = wa if k < 27 else wb
            rhs = sl[(ch, po + d0)][
                :, 16 * hf + d1:16 * hf + d1 + 16, d2:d2 + 32]
            nc.tensor.matmul(ps[:], wt_[:, k % 27, :], rhs,
                             start=(k == 0), stop=(k == 53))
        xpo = outsp.tile([128, 512], F32, tag="xpo")
        nc.vector.tensor_scalar_add(xpo[:], ps[:], cbt[:, cot:cot + 1])
        n0 = po * 1024 + hf * 512
        nc.sync.dma_start(xpd[b][cot][:, n0:n0 + 512], xpo[:])

    def phase2_pair(b, p):
        if p == 0:
            st2[b] = dict(
                egt=[egtp.tile([128, NLOC], BF16, tag="egt",
                               name=f"egt_{b}_{g_}") for g_ in range(GROUPS)],
                eps=[psE.tile([128, 512], F32, tag="eps",
                              name=f"eps_{b}_{g_}") for g_ in range(GROUPS)])
        egt, eps = st2[b]["egt"], st2[b]["eps"]
        # xp chunks for both j's of the pair, all 4 cots (used by logits
        # lhsT as F32 and by the PE transposes as F32R bitcast)
        xc = {}
        for dj in range(2):
            j = 2 * p + dj
            for cot in range(4):
                xt_ = xpcp.tile([128, 128], F32, tag="xpc")
                nc.sync.dma_start(
                    xt_[:], xpd[b][cot][:, 128 * j:128 * j + 128])
                xc[(dj, cot)] = xt_
        ests = []
        for g in range(GROUPS):
            ps = psL.tile([128, 2, 132], F32, tag="lps")
            for dj in range(2):
                for a in range(2):
                    nc.tensor.matmul(ps[:, dj, 66 * a:66 * a + 66],
                                     xc[(dj, 2 * g + a)][:], w2t[:],
                                     start=True, stop=True)
            lts = ltsp.tile([128, 2, 4, 33], F32, tag="lts")
            nc.vector.tensor_tensor(
                lts[:], ps[:].rearrange("p a (b c) -> p a b c", b=4, c=33),
                biasbc[:], ALU.add)
            tt = small.tile([128, 2, 4], F32, tag="tt")
            nc.vector.tensor_scalar(tt[:], lts[:, :, :, 32],
                                    0.4, -0.4, ALU.min, ALU.max)
            nc.vector.tensor_scalar_add(tt[:], tt[:], 0.5)
            rt = small.tile([128, 2, 4], F32, tag="rt")
            nc.vector.reciprocal(rt[:], tt[:])
            est = estp.tile([128, 2, 4, 32], F32R, tag="est")
            nc.vector.tensor_tensor(
                lts[:, :, :, 0:32], lts[:, :, :, 0:32],
                rt[:].to_broadcast((128, 2, 4, 32)), ALU.mult)
            nc.scalar.activation(est[:], lts[:, :, :, 0:32],
                                 ACTF.Exp, bias=m0b[:], scale=1.0)
            ests.append(est)
        for dj in range(2):
            j = 2 * p + dj
            xpt = xptp.tile([128, 512], F32R, tag="xpt")
            for cot in range(4):
                pt = psT.tile([128, 128], F32R, tag="tps")
                nc.tensor.transpose(pt[:], xc[(dj, cot)][:].bitcast(F32R),
                                    identr[:])
                nc.scalar.activation(xpt[:, 128 * cot:128 * cot + 128], pt[:],
                                     ACTF.Copy)
            for g in range(GROUPS):
                echunk = ests[g][:, dj, :, :].rearrange("p b c -> p (b c)")
                pe_t = psT.tile([128, 128], F32R, tag="tps")
                nc.tensor.transpose(pe_t[:], echunk, identr[:])
                nc.scalar.activation(egt[g][:, 128 * j:128 * j + 128], pe_t[:],
                                     ACTF.Copy)
                nc.tensor.matmul(eps[g][:], echunk, xpt[:],
                                 start=(p == 0 and dj == 0),
                                 stop=(p == 15 and dj == 1))

    def pack_ar(b):
        egt, eps = st2[b]["egt"], st2[b]["eps"]
        es2 = small.tile([128, 130], F32, tag="es2")
        for g in range(GROUPS):
            ssum = small.tile([128, 1], F32, tag="ssum")
            nc.vector.reduce_sum(ssum[:], egt[g][:], axis=AX.X)
            nc.vector.tensor_copy(es2[:, 65 * g + 64:65 * g + 65], ssum[:])
            for k in range(4):
                nc.vector.tensor_copy(
                    es2[32 * k:32 * k + 32, 65 * g:65 * g + 64],
                    eps[g][32 * k:32 * k + 32,
                           64 * (4 * g + k):64 * (4 * g + k) + 64])
        nc.sync.dma_start(es_in[b].ap(), es2[:])
        nc.gpsimd.collective_compute(
            "AllReduce", ALU.add,
            ins=[es_in[b].ap()], outs=[es_out[b].ap()],
            replica_groups=[list(range(NCORES))])

    def recon(b):
        egt = st2[b]["egt"]
        esr = small.tile([128, 130], F32, tag="esr")
        nc.sync.dma_start(esr[:], es_out[b].ap())
        fbs = []
        for g in range(GROUPS):
            r1 = small.tile([128, 1], F32, tag="r1")
            nc.vector.reciprocal(r1[:], esr[:, 65 * g + 64:65 * g + 65])
            ft1 = small.tile([128, 64], F32, tag="ft1")
            nc.vector.tensor_scalar_mul(ft1[:], esr[:, 65 * g:65 * g + 64],
                                        r1[:])
            ft = small.tile([128, 64], BF16, tag="ft")
            nc.vector.tensor_scalar(ft[:], ft1[:], r1[:], 1.0 / (1.0 + 1e-5),
                                    ALU.mult, ALU.mult)
            fb = small.tile([128, 128], BF16, tag="fb", name=f"fb_{b}_{g}")
            nc.vector.tensor_copy(fb[:], zerob[:])
            for a in range(2):
                nc.sync.dma_start(fb[64 * a:64 * a + 32, 0:64],
                                  ft[64 * a:64 * a + 32, :])
                nc.sync.dma_start(fb[64 * a + 32:64 * a + 64, 64:128],
                                  ft[64 * a + 32:64 * a + 64, :])
            fbs.append(fb)

        for w in range(8):
            oxs = []
            for pq in range(4):
                g, a = pq // 2, pq % 2
                pr = psA.tile([128, 512], F32, tag="cps")
                nc.tensor.matmul(pr[:], fbs[g][64 * a:64 * a + 64, :],
                                 egt[g][64 * a:64 * a + 64,
                                        512 * w:512 * w + 512],
                                 start=True, stop=True)
                ox = oxwp.tile([128, 512], F32R, tag="ox")
                nc.scalar.activation(ox[:], pr[:], ACTF.Copy)
                oxs.append(ox)
            for mt in range(2):
                po = psL.tile([128, 512], F32, tag="lps")
                for pq in range(4):
                    nc.tensor.matmul(po[:], owt[:, 2 * pq + mt, :],
                                     oxs[pq][:], start=(pq == 0),
                                     stop=(pq == 3))
                osb = outsp.tile([128, 512], F32, tag="osb")
                nc.vector.tensor_scalar_add(osb[:], po[:], obt[:, mt:mt + 1])
                nc.sync.dma_start(
                    out_d.ap()[b, 128 * mt:128 * mt + 128,
                               512 * w:512 * w + 512],
                    osb[:])

    # ---------------- interleaved schedule ---------------------------
    # conv groups are globally indexed gi = 32*b + g.  Actions fire after
    # the conv group with index == trigger has been emitted:
    #   phase2 pair p of b  @ 32b + 24 + p//2   (= when its xp chunks exist)
    #   slab prefetch b+1   @ 32b + 29
    #   pack+AllReduce b    @ 32b + 32
    #   recon b             @ 32b + 48          (AR has ~16 groups to land)
    events = []
    for b in range(B):
        for p in range(16):
            events.append((32 * b + 24 + p // 2, 2,
                           (lambda b=b, p=p: phase2_pair(b, p))))
        if b + 1 < B:
            events.append((32 * b + 31, 1, (lambda b=b: load_slabs(b + 1))))
        events.append((32 * b + 32, 3, (lambda b=b: pack_ar(b))))
        events.append((32 * b + 48, 4, (lambda b=b: recon(b))))
    events.sort(key=lambda e: (e[0], e[1]))

    load_weights(0)
    load_slabs(0)
    ei = 0
    for gi in range(32 * B):
        if gi % 8 == 4 and gi // 8 + 1 < 4 * B:
            load_weights(gi // 8 + 1)
        conv_group(gi // 32, gi % 32)
        while ei < len(events) and events[ei][0] <= gi:
            events[ei][2]()
            ei += 1
    while ei < len(events):
        events[ei][2]()
        ei += 1


def _prep_inputs(x, conv_w, conv_b, slice_w, slice_b, ada_w, ada_b, out_w, out_b):
    """Shard/transpose/pad the full inputs into 8 per-core input maps."""
    x = np.ascontiguousarray(x, np.float32)
    xT = np.zeros((C, B, 34, 34, 34), np.float32)
    xT[:, :, 1:33, 1:33, 1:33] = x.reshape(B, HH, HH, HH, C).transpose(4, 0, 1, 2, 3)

    # conv weights: [co, ci, 3,3,3] -> [ci%128, tap, ci//128, co]
    # [ci%128, k=(t*2+ch), cot, co%128]
    wst = np.ascontiguousarray(
        conv_w.reshape(INNER, C, 27).transpose(1, 2, 0)      # [ci, t, co]
              .reshape(2, 128, 27, 4, 128)                   # [ch, ci, t, cot, co]
              .transpose(1, 2, 0, 3, 4)                      # [ci, t, ch, cot, co]
              .reshape(128, 54, 4, 128), np.float32)
    cb = np.ascontiguousarray(conv_b.reshape(4, 128).T, np.float32)

    w2 = np.zeros((128, 66), np.float32)
    w2[0:64, 0:32] = slice_w.T
    w2[0:64, 32] = ada_w[0]
    w2[64:128, 33:65] = slice_w.T
    w2[64:128, 65] = ada_w[0]

    bvec = np.concatenate([slice_b - LNLE, ada_b]).astype(np.float32)  # [33]
    brow = np.tile(bvec, 8).reshape(1, 264)

    ow = np.ascontiguousarray(
        out_w.T.reshape(4, 128, 2, 128).transpose(1, 0, 2, 3), np.float32) \
        .reshape(128, 8, 128)
    ob = np.ascontiguousarray(out_b.reshape(2, 128).T, np.float32)

    in_maps = []
    for i in range(NCORES):
        slab = np.ascontiguousarray(xT[:, :, 4 * i:4 * i + 6, :, :]) \
            .reshape(2, 128, B, 6, 34, 34)
        in_maps.append({"xt": slab, "wst": wst, "cb": cb, "w2": w2,
                        "brow": brow, "ow": ow, "ob": ob})
    return in_maps


def kernel(**inputs):
    if "nc" not in _CACHE:
        _CACHE["nc"] = _build()
    nc = _CACHE["nc"]
    in_maps = _prep_inputs(
        np.asarray(inputs["x"]), np.asarray(inputs["conv_w"]),
        np.asarray(inputs["conv_b"]), np.asarray(inputs["slice_w"]),
        np.asarray(inputs["slice_b"]), np.asarray(inputs["ada_w"]),
        np.asarray(inputs["ada_b"]), np.asarray(inputs["out_w"]),
        np.asarray(inputs["out_b"]))
    res = run_bass_kernel_spmd(nc, in_maps, core_ids=list(range(NCORES)))
    out = np.empty((B, 32768, 256), np.float32)
    for i in range(NCORES):
        o = res.results[i]["out"]            # [B, 256, 4096]
        out[:, 4096 * i:4096 * (i + 1), :] = o.transpose(0, 2, 1)
    return out


# revision 8
# speedup vs baseline: 1.2720x; 1.0134x over previous
"""Physics-Attention Structured Mesh 3D — Trainium2 Bass kernel, 8 NeuronCores.

Sharding: spatial-parallel over the first grid axis (s0: 32 planes -> 8 slabs
of 4 planes each, halo 1 plane per side, zero-padded on the host). The
point-axis softmax is made shift-invariant with a static exponent shift M0,
so the only cross-core traffic is one AllReduce-add of the (E, S)
accumulators (66 KB) per batch element.

v2: the emission interleaves the post-conv phases (logits/temp/exp,
transposes, E accumulation, AllReduce, reconstruction) between the conv's
PSUM accumulation groups so the PE instruction stream never drains. This
keeps the HAM clock gate at its sustained level (13/16) instead of
dropping to 4/8 during the mixed phases, and hides the AllReduce latency
under the next batch's conv. PSUM->SBUF copies ride the scalar engine
(otherwise-idle) so the vector engine keeps up inside the interleave
windows.

Per core pipeline (per batch element b):
  conv   : 27-tap shifted matmul, fp32r, 32 PSUM groups -> xp [512, 4096]
  logits : xp chunks (fp32) @ [slice_w|ada_w] -> lt wide tiles (n-layout)
  temp   : t = 0.5+clip(v), lt = l * recip(t)   (fp32 DVE)
  exp    : e = exp(lt - M0) (ACT) -> fp32r
  E/S    : E = e^T @ xp^T (PE transposes + fp32r matmuls), S = sum_n e
  AllReduce-add(E||S) ; F = E * recip(S)^2 / (1+1e-5)   (bf16)
  recon  : out_x = F_blkdiag @ e_g (bf16) ; proj: out = out_x @ out_w^T + out_b
"""
import numpy as np

import concourse.bacc as bacc
import concourse.mybir as mybir
import concourse.tile as tile
from concourse.bass_utils import run_bass_kernel_spmd
from concourse.masks import make_identity

F32 = mybir.dt.float32
F32R = mybir.dt.float32r
BF16 = mybir.dt.bfloat16
ALU = mybir.AluOpType
AX = mybir.AxisListType
ACTF = mybir.ActivationFunctionType

NCORES = 8
B = 4
C = 256
INNER = 512
HH = 32
NLOC = 4 * HH * HH          # 4096 points per core
NCH = NLOC // 128           # 32 n-chunks
GROUPS = 2                  # co-tile pair-groups
M0 = 60.0                   # static softmax exponent shift
LNLE = float(np.log(-np.log(np.float32(1e-6))))

_CACHE = {}


def _build():
    nc = bacc.Bacc("TRN2", target_bir_lowering=False, debug=False,
                   num_devices=NCORES)

    xt_d = nc.declare_dram_parameter("xt", [2, 128, B, 6, 34, 34], F32, isOutput=False)
    wst_d = nc.declare_dram_parameter("wst", [128, 54, 4, 128], F32, isOutput=False)
    cb_d = nc.declare_dram_parameter("cb", [128, 4], F32, isOutput=False)
    w2_d = nc.declare_dram_parameter("w2", [128, 66], F32, isOutput=False)
    brow_d = nc.declare_dram_parameter("brow", [1, 264], F32, isOutput=False)
    ow_d = nc.declare_dram_parameter("ow", [128, 8, 128], F32, isOutput=False)
    ob_d = nc.declare_dram_parameter("ob", [128, 2], F32, isOutput=False)
    out_d = nc.declare_dram_parameter("out", [B, 256, NLOC], F32, isOutput=True)

    es_in = [nc.dram_tensor(f"es_in{b}", [128, 130], F32) for b in range(B)]
    es_out = [nc.dram_tensor(f"es_out{b}", [128, 130], F32, addr_space="Shared")
              for b in range(B)]
    env = dict(xt_d=xt_d, wst_d=wst_d, cb_d=cb_d, w2_d=w2_d, brow_d=brow_d,
               ow_d=ow_d, ob_d=ob_d, out_d=out_d, es_in=es_in, es_out=es_out)

    with tile.TileContext(nc) as tc, \
         tc.tile_pool(name="konst", bufs=1) as konst, \
         tc.tile_pool(name="wstr", bufs=2) as wstrp, \
         tc.tile_pool(name="slab", bufs=12) as slabp, \
         tc.tile_pool(name="lts", bufs=4) as ltsp, \
         tc.tile_pool(name="est", bufs=4) as estp, \
         tc.tile_pool(name="egt", bufs=4) as egtp, \
         tc.tile_pool(name="xpc", bufs=16) as xpcp, \
         tc.tile_pool(name="xpt", bufs=2) as xptp, \
         tc.tile_pool(name="small", bufs=4) as small, \
         tc.tile_pool(name="oxw", bufs=4) as oxwp, \
         tc.tile_pool(name="outs", bufs=3) as outsp, \
         tc.tile_pool(name="dram", bufs=8, space="DRAM") as dramp, \
         tc.tile_pool(name="psA", bufs=2, space="PSUM") as psA, \
         tc.tile_pool(name="psL", bufs=2, space="PSUM") as psL, \
         tc.tile_pool(name="psT", bufs=2, space="PSUM") as psT, \
         tc.tile_pool(name="psE", bufs=2, space="PSUM") as psE:

        pools = dict(konst=konst, wstrp=wstrp, slabp=slabp, ltsp=ltsp,
                     estp=estp, egtp=egtp, xpcp=xpcp, xptp=xptp, small=small,
                     oxwp=oxwp, outsp=outsp, dramp=dramp, psA=psA, psL=psL,
                     psT=psT, psE=psE)
        _emit(nc, tc, env, pools)

    nc.compile()
    return nc


def _emit(nc, tc, env, P):
    xt_d, wst_d, cb_d, w2_d, brow_d = (env[k] for k in
        ("xt_d", "wst_d", "cb_d", "w2_d", "brow_d"))
    ow_d, ob_d, out_d, es_in, es_out = (env[k] for k in
        ("ow_d", "ob_d", "out_d", "es_in", "es_out"))
    konst, wstrp, slabp, ltsp, estp = (P[k] for k in
        ("konst", "wstrp", "slabp", "ltsp", "estp"))
    egtp, xpcp, xptp, small, oxwp, outsp, dramp = (P[k] for k in
        ("egtp", "xpcp", "xptp", "small", "oxwp", "outsp", "dramp"))
    psA, psL, psT, psE = (P[k] for k in ("psA", "psL", "psT", "psE"))

    # ---- first conv group's operands go to the DMA queue first, so the
    # PE can start ~15us earlier; constants follow (none are needed until
    # the first conv drain / first phase2 pair, long after) ----
    wa0 = wstrp.tile([128, 27, 128], F32R, tag="wa", name="wa_0_0")
    nc.sync.dma_start(wa0[:], wst_d.ap()[:, 0:27, 0, :].bitcast(F32R))
    sl0 = {}
    for pl in range(3):
        for ch in range(2):
            st0 = slabp.tile([128, 34, 34], F32R, tag="sl",
                             name=f"sl_0_{ch}_{pl}")
            nc.sync.dma_start(st0[:], xt_d.ap()[ch, :, 0, pl, :, :].bitcast(F32R))
            sl0[(ch, pl)] = st0
    wb0 = wstrp.tile([128, 27, 128], F32R, tag="wb", name="wb_0_0")
    nc.sync.dma_start(wb0[:], wst_d.ap()[:, 27:54, 0, :].bitcast(F32R))
    for pl in range(3, 6):
        for ch in range(2):
            st0 = slabp.tile([128, 34, 34], F32R, tag="sl",
                             name=f"sl_0_{ch}_{pl}")
            nc.sync.dma_start(st0[:], xt_d.ap()[ch, :, 0, pl, :, :].bitcast(F32R))
            sl0[(ch, pl)] = st0

    # ---- constants ----
    cbt = konst.tile([128, 4], F32, tag="cbt")
    nc.sync.dma_start(cbt[:], cb_d.ap())
    w2t = konst.tile([128, 66], F32, tag="w2t")
    nc.sync.dma_start(w2t[:], w2_d.ap())
    biasbc = konst.tile([128, 2, 4, 33], F32, tag="biasbc")
    nc.sync.dma_start(biasbc[:].rearrange("p a b c -> p (a b c)"),
                      brow_d.ap().to_broadcast((128, 264)))
    ident = konst.tile([128, 128], F32, tag="ident")
    make_identity(nc, ident)
    identr = konst.tile([128, 128], F32R, tag="identr")
    nc.vector.tensor_copy(identr[:], ident[:])
    owt = konst.tile([128, 8, 128], F32R, tag="owt")
    nc.sync.dma_start(owt[:], ow_d.ap().bitcast(F32R))
    obt = konst.tile([128, 2], F32, tag="obt")
    nc.sync.dma_start(obt[:], ob_d.ap())
    m0b = konst.tile([128, 1], F32, tag="m0b")
    nc.vector.memset(m0b[:], -M0)
    zerob = konst.tile([128, 128], BF16, tag="zerob")
    nc.vector.memset(zerob[:], 0.0)

    xpd = [[dramp.tile([128, NLOC], F32, tag="xpd", name=f"xpd_{b_}_{c_}")
            for c_ in range(4)] for b_ in range(B)]

    # ---------------- emission helpers -------------------------------
    wtab = {}    # cot_global -> (wa, wb)
    sltab = {}   # b -> {(ch, pl): slab tile}
    st2 = {}     # b -> dict(egt=[...], eps=[...])

    def load_weights(cg):
        b_, cot = cg // 4, cg % 4
        wa = wstrp.tile([128, 27, 128], F32R, tag="wa", name=f"wa_{b_}_{cot}")
        nc.sync.dma_start(wa[:], wst_d.ap()[:, 0:27, cot, :].bitcast(F32R))
        wb = wstrp.tile([128, 27, 128], F32R, tag="wb", name=f"wb_{b_}_{cot}")
        nc.sync.dma_start(wb[:], wst_d.ap()[:, 27:54, cot, :].bitcast(F32R))
        wtab[cg] = (wa, wb)

    def load_slabs(b):
        sl = {}
        for pl in range(6):
            for ch in range(2):
                st = slabp.tile([128, 34, 34], F32R, tag="sl",
                                name=f"sl_{b}_{ch}_{pl}")
                nc.sync.dma_start(
                    st[:], xt_d.ap()[ch, :, b, pl, :, :].bitcast(F32R))
                sl[(ch, pl)] = st
        sltab[b] = sl

    def conv_group(b, g):
        cot, po, hf = g // 8, (g % 8) // 2, g % 2
        wa, wb = wtab[4 * b + cot]
        sl = sltab[b]
        ps = psA.tile([128, 512], F32, tag="cps")
        for k in range(54):
            t, ch = k // 2, k % 2
            d0, d1, d2 = t // 9, (t // 3) % 3, t % 3
            wt_ = wa if k < 27 else wb
            rhs = sl[(ch, po + d0)][
                :, 16 * hf + d1:16 * hf + d1 + 16, d2:d2 + 32]
            nc.tensor.matmul(ps[:], wt_[:, k % 27, :], rhs,
                             start=(k == 0), stop=(k == 53))
        xpo = outsp.tile([128, 512], F32, tag="xpo")
        nc.vector.tensor_scalar_add(xpo[:], ps[:], cbt[:, cot:cot + 1])
        n0 = po * 1024 + hf * 512
        nc.sync.dma_start(xpd[b][cot][:, n0:n0 + 512], xpo[:])

    def phase2_pair(b, p):
        if p == 0:
            st2[b] = dict(
                egt=[egtp.tile([128, NLOC], BF16, tag="egt",
                               name=f"egt_{b}_{g_}") for g_ in range(GROUPS)],
                eps=[psE.tile([128, 512], F32, tag="eps",
                              name=f"eps_{b}_{g_}") for g_ in range(GROUPS)],
                sac=small.tile([128, 2], F32, tag="sac", bufs=2,
                               name=f"sac_{b}"))
        egt, eps, sac = st2[b]["egt"], st2[b]["eps"], st2[b]["sac"]
        # xp chunks for both j's of the pair, all 4 cots (used by logits
        # lhsT as F32 and by the PE transposes as F32R bitcast)
        xc = {}
        for dj in range(2):
            j = 2 * p + dj
            for cot in range(4):
                xt_ = xpcp.tile([128, 128], F32, tag="xpc")
                nc.sync.dma_start(
                    xt_[:], xpd[b][cot][:, 128 * j:128 * j + 128])
                xc[(dj, cot)] = xt_
        ests = []
        for g in range(GROUPS):
            ps = psL.tile([128, 2, 132], F32, tag="lps")
            for dj in range(2):
                for a in range(2):
                    nc.tensor.matmul(ps[:, dj, 66 * a:66 * a + 66],
                                     xc[(dj, 2 * g + a)][:], w2t[:],
                                     start=True, stop=True)
            lts = ltsp.tile([128, 2, 4, 33], F32, tag="lts")
            nc.vector.tensor_tensor(
                lts[:], ps[:].rearrange("p a (b c) -> p a b c", b=4, c=33),
                biasbc[:], ALU.add)
            tt = small.tile([128, 2, 4], F32, tag="tt")
            nc.vector.tensor_scalar(tt[:], lts[:, :, :, 32],
                                    0.4, -0.4, ALU.min, ALU.max)
            nc.vector.tensor_scalar_add(tt[:], tt[:], 0.5)
            rt = small.tile([128, 2, 4], F32, tag="rt")
            nc.vector.reciprocal(rt[:], tt[:])
            est = estp.tile([128, 2, 4, 32], F32R, tag="est")
            nc.vector.tensor_tensor(
                lts[:, :, :, 0:32], lts[:, :, :, 0:32],
                rt[:].to_broadcast((128, 2, 4, 32)), ALU.mult)
            nc.scalar.activation(est[:], lts[:, :, :, 0:32],
                                 ACTF.Exp, bias=m0b[:], scale=1.0)
            ests.append(est)
        for dj in range(2):
            j = 2 * p + dj
            xpt = xptp.tile([128, 512], F32R, tag="xpt")
            for cot in range(4):
                pt = psT.tile([128, 128], F32R, tag="tps")
                nc.tensor.transpose(pt[:], xc[(dj, cot)][:].bitcast(F32R),
                                    identr[:])
                nc.scalar.activation(xpt[:, 128 * cot:128 * cot + 128], pt[:],
                                     ACTF.Copy)
            for g in range(GROUPS):
                echunk = ests[g][:, dj, :, :].rearrange("p b c -> p (b c)")
                pe_t = psT.tile([128, 128], F32R, tag="tps")
                nc.tensor.transpose(pe_t[:], echunk, identr[:])
                nc.scalar.activation(egt[g][:, 128 * j:128 * j + 128], pe_t[:],
                                     ACTF.Copy)
                nc.tensor.matmul(eps[g][:], echunk, xpt[:],
                                 start=(p == 0 and dj == 0),
                                 stop=(p == 15 and dj == 1))
        # incremental S: fold this pair's 256 columns of e^T into sac so
        # the final pack doesn't pay a full [128, 4096] reduction
        for g in range(GROUPS):
            if p == 0:
                nc.vector.reduce_sum(sac[:, g:g + 1],
                                     egt[g][:, 256 * p:256 * p + 256],
                                     axis=AX.X)
            else:
                spp = small.tile([128, 1], F32, tag="spp")
                nc.vector.reduce_sum(spp[:], egt[g][:, 256 * p:256 * p + 256],
                                     axis=AX.X)
                nc.vector.tensor_tensor(sac[:, g:g + 1], sac[:, g:g + 1],
                                        spp[:], ALU.add)

    def pack_ar(b):
        eps, sac = st2[b]["eps"], st2[b]["sac"]
        es2 = small.tile([128, 130], F32, tag="es2")
        for g in range(GROUPS):
            nc.vector.tensor_copy(es2[:, 65 * g + 64:65 * g + 65],
                                  sac[:, g:g + 1])
            for k in range(4):
                nc.vector.tensor_copy(
                    es2[32 * k:32 * k + 32, 65 * g:65 * g + 64],
                    eps[g][32 * k:32 * k + 32,
                           64 * (4 * g + k):64 * (4 * g + k) + 64])
        nc.sync.dma_start(es_in[b].ap(), es2[:])
        nc.gpsimd.collective_compute(
            "AllReduce", ALU.add,
            ins=[es_in[b].ap()], outs=[es_out[b].ap()],
            replica_groups=[list(range(NCORES))])

    def recon(b):
        egt = st2[b]["egt"]
        esr = small.tile([128, 130], F32, tag="esr")
        nc.sync.dma_start(esr[:], es_out[b].ap())
        fbs = []
        for g in range(GROUPS):
            r1 = small.tile([128, 1], F32, tag="r1")
            nc.vector.reciprocal(r1[:], esr[:, 65 * g + 64:65 * g + 65])
            ft1 = small.tile([128, 64], F32, tag="ft1")
            nc.vector.tensor_scalar_mul(ft1[:], esr[:, 65 * g:65 * g + 64],
                                        r1[:])
            ft = small.tile([128, 64], BF16, tag="ft")
            nc.vector.tensor_scalar(ft[:], ft1[:], r1[:], 1.0 / (1.0 + 1e-5),
                                    ALU.mult, ALU.mult)
            fb = small.tile([128, 128], BF16, tag="fb", name=f"fb_{b}_{g}")
            nc.vector.tensor_copy(fb[:], zerob[:])
            for a in range(2):
                nc.sync.dma_start(fb[64 * a:64 * a + 32, 0:64],
                                  ft[64 * a:64 * a + 32, :])
                nc.sync.dma_start(fb[64 * a + 32:64 * a + 64, 64:128],
                                  ft[64 * a + 32:64 * a + 64, :])
            fbs.append(fb)

        for w in range(8):
            oxs = []
            for pq in range(4):
                g, a = pq // 2, pq % 2
                pr = psA.tile([128, 512], F32, tag="cps")
                nc.tensor.matmul(pr[:], fbs[g][64 * a:64 * a + 64, :],
                                 egt[g][64 * a:64 * a + 64,
                                        512 * w:512 * w + 512],
                                 start=True, stop=True)
                ox = oxwp.tile([128, 512], F32R, tag="ox")
                nc.scalar.activation(ox[:], pr[:], ACTF.Copy)
                oxs.append(ox)
            for mt in range(2):
                po = psL.tile([128, 512], F32, tag="lps")
                for pq in range(4):
                    nc.tensor.matmul(po[:], owt[:, 2 * pq + mt, :],
                                     oxs[pq][:], start=(pq == 0),
                                     stop=(pq == 3))
                osb = outsp.tile([128, 512], F32, tag="osb")
                nc.vector.tensor_scalar_add(osb[:], po[:], obt[:, mt:mt + 1])
                nc.sync.dma_start(
                    out_d.ap()[b, 128 * mt:128 * mt + 128,
                               512 * w:512 * w + 512],
                    osb[:])

    # ---------------- interleaved schedule ---------------------------
    # conv groups are globally indexed gi = 32*b + g.  Actions fire after
    # the conv group with index == trigger has been emitted:
    #   phase2 pair p of b  @ 32b + 24 + p//2   (= when its xp chunks exist)
    #   slab prefetch b+1   @ 32b + 29
    #   pack+AllReduce b    @ 32b + 32
    #   recon b             @ 32b + 48          (AR has ~16 groups to land)
    events = []
    for b in range(B):
        for p in range(16):
            events.append((32 * b + 24 + p // 2, 2,
                           (lambda b=b, p=p: phase2_pair(b, p))))
        if b + 1 < B:
            events.append((32 * b + 31, 1, (lambda b=b: load_slabs(b + 1))))
        events.append((32 * b + 32, 3, (lambda b=b: pack_ar(b))))
        events.append((32 * b + 48, 4, (lambda b=b: recon(b))))
    events.sort(key=lambda e: (e[0], e[1]))

    wtab[0] = (wa0, wb0)
    sltab[0] = sl0
    ei = 0
    for gi in range(32 * B):
        if gi % 8 == 4 and gi // 8 + 1 < 4 * B:
            load_weights(gi // 8 + 1)
        conv_group(gi // 32, gi % 32)
        while ei < len(events) and events[ei][0] <= gi:
            events[ei][2]()
            ei += 1
    while ei < len(events):
        events[ei][2]()
        ei += 1


def _prep_inputs(x, conv_w, conv_b, slice_w, slice_b, ada_w, ada_b, out_w, out_b):
    """Shard/transpose/pad the full inputs into 8 per-core input maps."""
    x = np.ascontiguousarray(x, np.float32)
    xT = np.zeros((C, B, 34, 34, 34), np.float32)
    xT[:, :, 1:33, 1:33, 1:33] = x.reshape(B, HH, HH, HH, C).transpose(4, 0, 1, 2, 3)

    # conv weights: [co, ci, 3,3,3] -> [ci%128, tap, ci//128, co]
    # [ci%128, k=(t*2+ch), cot, co%128]
    wst = np.ascontiguousarray(
        conv_w.reshape(INNER, C, 27).transpose(1, 2, 0)      # [ci, t, co]
              .reshape(2, 128, 27, 4, 128)                   # [ch, ci, t, cot, co]
              .transpose(1, 2, 0, 3, 4)                      # [ci, t, ch, cot, co]
              .reshape(128, 54, 4, 128), np.float32)
    cb = np.ascontiguousarray(conv_b.reshape(4, 128).T, np.float32)

    w2 = np.zeros((128, 66), np.float32)
    w2[0:64, 0:32] = slice_w.T
    w2[0:64, 32] = ada_w[0]
    w2[64:128, 33:65] = slice_w.T
    w2[64:128, 65] = ada_w[0]

    bvec = np.concatenate([slice_b - LNLE, ada_b]).astype(np.float32)  # [33]
    brow = np.tile(bvec, 8).reshape(1, 264)

    ow = np.ascontiguousarray(
        out_w.T.reshape(4, 128, 2, 128).transpose(1, 0, 2, 3), np.float32) \
        .reshape(128, 8, 128)
    ob = np.ascontiguousarray(out_b.reshape(2, 128).T, np.float32)

    in_maps = []
    for i in range(NCORES):
        slab = np.ascontiguousarray(xT[:, :, 4 * i:4 * i + 6, :, :]) \
            .reshape(2, 128, B, 6, 34, 34)
        in_maps.append({"xt": slab, "wst": wst, "cb": cb, "w2": w2,
                        "brow": brow, "ow": ow, "ob": ob})
    return in_maps


def kernel(**inputs):
    if "nc" not in _CACHE:
        _CACHE["nc"] = _build()
    nc = _CACHE["nc"]
    in_maps = _prep_inputs(
        np.asarray(inputs["x"]), np.asarray(inputs["conv_w"]),
        np.asarray(inputs["conv_b"]), np.asarray(inputs["slice_w"]),
        np.asarray(inputs["slice_b"]), np.asarray(inputs["ada_w"]),
        np.asarray(inputs["ada_b"]), np.asarray(inputs["out_w"]),
        np.asarray(inputs["out_b"]))
    res = run_bass_kernel_spmd(nc, in_maps, core_ids=list(range(NCORES)))
    out = np.empty((B, 32768, 256), np.float32)
    for i in range(NCORES):
        o = res.results[i]["out"]            # [B, 256, 4096]
        out[:, 4096 * i:4096 * (i + 1), :] = o.transpose(0, 2, 1)
    return out
